# revision 54
# baseline (speedup 1.0000x reference)
"""Trainium2 Bass kernel for nn_FChCombxValEncoder (HDC n-gram encoder).

Computation: idx = quantize(x) -> signal = signals_weight[idx] -> bind with
feat_weight -> 4-gram product with per-step D-rolls -> bundle sum -> sign.

Distribution: feature axis (4096) sharded across 8 cores (512 n-gram starts
each); per-core bundle partials are AllReduced in 3 pipelined D-slices that
fire as soon as their chunks finish, hiding the collective latency behind
later-chunk compute (a single end-of-kernel AllReduce costs ~47us exposed).

Layout: each partition p holds FOUR consecutive rows t = 4p+j as four
streams side by side in the free dim (stream pitch 2005).  Per 2000-wide
D-chunk:
    S_j = sig_j . feat_j          one strided in-place bind op (bf16 2x)
    U_j = S_j . S_{j+1}(+1)       j=0..2 one strided op; j=3 uses A1 =
                                  S_0 of partition p+1 via split SBUF DMA
    Q_j = U_j . U_{j+2}(+2)       two strided in-place ops; j=2,3 use
                                  V = U_{0,1}[p+1] via shift-matmul on PE
                                  (zero col 127 => exact for +/-1)
    R   = ones^T @ Q              PSUM accumulate, 512-col segs
The 3 grams per core needing rows past the 512-row pack ("orphans") are
computed ONCE for all 5 chunks on 30 partitions (6 rows x 5 chunks); the
per-chunk selection happens in the reduce matmul via a 30x5 0/1 matrix E
(garbage rows weighted 0; unwritten rows zeroed once).

Sign+output for each slice is emitted only after the chunk loop so its
AllReduce-wait never blocks the Vector queue ahead of chunk work; the
AllReduce triggers themselves are emitted one chunk late so the GpSimd
queue never stalls on a trigger semaphore ahead of gather dispatches.

All values are +/-1 so bf16 is exact; bundle partials are integers < 2^24
so fp32 PSUM/AllReduce is exact; the output sign never sees zero (4093 odd
terms).  Index quantization is bit-exact via a host fp32 threshold table:
idx = #{k : x >= b_k}.
"""
import sys

sys.path.insert(0, "/opt/trn_rl_repo")

import numpy as np
import ml_dtypes

import concourse.bass as bass
import concourse.bacc as bacc
import concourse.tile as tile
import concourse.mybir as mybir
from concourse.bass_utils import run_bass_kernel_spmd

# ---- problem constants ----
MAX_VAL = 52000.0
MIN_VAL = -53000.0
NUM_LEVELS = 1000
NGRAM = 4
D = 10000
NFEAT = 4096
NCORE = 8

PER_CORE = 512
ROLL = NGRAM - 1

W = 2000                      # D-chunk width
NCHUNK = D // W               # 5
PS = 2005                     # packed stream pitch
WB = 2003                     # S cols needed per chunk (U reach)
UP = 2004                     # U stream pitch (even), data width 2002
WU = 2002                     # U stream data width
DPAD = D + 5                  # table/orphan row width 10005
NORPH = 6                     # orphan rows per core (t = 509..514)
NORPH_T = NCHUNK * NORPH      # 30: orphan rows unrolled over chunks
SEGS = [(0, 512), (512, 1024), (1024, 1536), (1536, 2000)]
# AllReduce D-slices: (start, end, last chunk needed)
AR_SLICES = [(0, 6000, 2), (6000, 10000, 4)]

F32 = mybir.dt.float32
BF16 = mybir.dt.bfloat16
I32 = mybir.dt.int32
_BF = ml_dtypes.bfloat16


# ---------------------------------------------------------------- host prep
def _f2o(u):
    b = u.view(np.uint32).astype(np.int64)
    return np.where(b < 0x80000000, b + 0x80000000, 0xFFFFFFFF - b)


def _o2f(o):
    b = np.where(o >= 0x80000000, o - 0x80000000, 0xFFFFFFFF - o).astype(np.uint64)
    return b.astype(np.uint32).view(np.float32)


def _g(v):
    v = v.astype(np.float32)
    t = (v - np.float32(MIN_VAL)).astype(np.float32)
    t = (t / np.float32(MAX_VAL - MIN_VAL)).astype(np.float32)
    t = (t * np.float32(NUM_LEVELS - 1)).astype(np.float32)
    return np.clip(np.round(t), 0.0, float(NUM_LEVELS - 1))


def _thresholds():
    ks = np.arange(1, NUM_LEVELS, dtype=np.float32)
    lo = _f2o(np.full(ks.shape, np.float32(MIN_VAL) - np.float32(2.0)))
    hi = _f2o(np.full(ks.shape, np.float32(MAX_VAL) + np.float32(2.0)))
    for _ in range(64):
        mid = (lo + hi) // 2
        ge = _g(_o2f(mid)) >= ks
        hi = np.where(ge, mid, hi)
        lo = np.where(ge, lo, mid)
        if np.all(hi - lo <= 1):
            break
    return _o2f(hi)


_CACHE = {}


def _host_constants():
    if "thr" not in _CACHE:
        _CACHE["thr"] = np.tile(_thresholds()[None, :], (128, 1)).astype(np.float32)
        sh1 = np.zeros((128, 128), dtype=_BF)
        for i in range(127):
            sh1[i + 1, i] = 1.0          # V[m] = U[m+1]; col 127 zero
        _CACHE["sh1"] = sh1
        _CACHE["ones_red"] = np.ones((128, 1), dtype=_BF)
        _CACHE["zrow"] = np.zeros((1, PS), dtype=_BF)
        eo = np.zeros((NORPH_T, NCHUNK), dtype=_BF)
        for c in range(NCHUNK):
            eo[NORPH * c:NORPH * c + 3, c] = 1.0
        _CACHE["eorph"] = eo
    return _CACHE


# ---------------------------------------------------------------- program
def _build_program():
    nc = bacc.Bacc("TRN2", target_bir_lowering=False, debug=False,
                   num_devices=NCORE)

    x4_d = nc.dram_tensor("x4", (128, 4), F32, kind="ExternalInput")
    xo_d = nc.dram_tensor("x_orph", (NORPH, 1), F32, kind="ExternalInput")
    thr_d = nc.dram_tensor("thr", (128, NUM_LEVELS - 1), F32, kind="ExternalInput")
    table_d = nc.dram_tensor("table", (NUM_LEVELS, DPAD), BF16, kind="ExternalInput")
    feat4_d = nc.dram_tensor("feat4", (NCHUNK, 128, 4 * PS), BF16, kind="ExternalInput")
    feato_d = nc.dram_tensor("feat_orph", (NORPH_T, PS), BF16, kind="ExternalInput")
    sh1_d = nc.dram_tensor("sh1", (128, 128), BF16, kind="ExternalInput")
    eorph_d = nc.dram_tensor("eorph", (NORPH_T, NCHUNK), BF16, kind="ExternalInput")
    onr_d = nc.dram_tensor("ones_red", (128, 1), BF16, kind="ExternalInput")
    zrow_d = nc.dram_tensor("zrow", (1, PS), BF16, kind="ExternalInput")
    out_d = nc.dram_tensor("out", (1, D), F32, kind="ExternalOutput")

    cc_in = nc.dram_tensor("cc_in", (1, D), F32)
    cc_out = nc.dram_tensor("cc_out", (1, D), F32, addr_space="Shared")

    NTH = NUM_LEVELS - 1

    # raw double-buffered A' tensors; row 127 kept zero
    a1_raw = [nc.alloc_sbuf_tensor(f"a1f{i}", [128, WU], BF16).ap()
              for i in range(2)]

    with tile.TileContext(nc) as tc:
        with tc.tile_pool(name="const", bufs=1) as cpool, \
             tc.tile_pool(name="scr", bufs=1) as spool_s, \
             tc.tile_pool(name="loads", bufs=3) as lpool, \
             tc.tile_pool(name="work", bufs=2) as wpool, \
             tc.tile_pool(name="orph", bufs=1) as opool, \
             tc.tile_pool(name="vone", bufs=1) as vpool, \
             tc.tile_pool(name="fin", bufs=1) as fpool, \
             tc.tile_pool(name="pv", bufs=4, space="PSUM") as pv, \
             tc.tile_pool(name="pacc", bufs=1, space="PSUM") as pacc:

            # ---- const loads; thr split across two queues ----
            thr = cpool.tile([128, NTH], F32)
            for k in range(2):
                [nc.sync, nc.scalar][k].dma_start(
                    out=thr[64 * k:64 * (k + 1), :],
                    in_=thr_d[64 * k:64 * (k + 1), :])
            sh1 = cpool.tile([128, 128], BF16)
            nc.sync.dma_start(out=sh1[:, :], in_=sh1_d[:, :])
            onr = cpool.tile([128, 1], BF16)
            nc.sync.dma_start(out=onr[:, :], in_=onr_d[:, :])
            eorph = cpool.tile([NORPH_T, NCHUNK], BF16)
            nc.sync.dma_start(out=eorph[:, :], in_=eorph_d[:, :])
            x4 = cpool.tile([128, 4], F32)
            nc.sync.dma_start(out=x4[:, :], in_=x4_d[:, :])
            xo = cpool.tile([NORPH, 1], F32)
            nc.scalar.dma_start(out=xo[:, :], in_=xo_d[:, :])

            for a1 in a1_raw:
                nc.sync.dma_start(out=a1[127:128, 0:WU], in_=zrow_d[0:1, 0:WU])

            # ---- orphan indices first so orphan gathers dispatch early ----
            ge = spool_s.tile([128, NTH], BF16, tag="ge")
            idxf = spool_s.tile([128, 1], F32, tag="idxf")
            nc.vector.tensor_scalar(
                out=ge[0:NORPH, :], in0=thr[0:NORPH, :], scalar1=xo[:, 0:1],
                scalar2=None, op0=mybir.AluOpType.is_le)
            nc.vector.tensor_reduce(out=idxf[0:NORPH, :], in_=ge[0:NORPH, :],
                                    axis=mybir.AxisListType.X,
                                    op=mybir.AluOpType.add)
            idxo = cpool.tile([NORPH, 1], I32, tag="idxo")
            nc.vector.tensor_copy(out=idxo[:, :], in_=idxf[0:NORPH, :])

            # ---- orphan gathers (6 rows x 5 chunks, chunk-major) ----
            sig_o = opool.tile([NORPH_T, PS], BF16, tag="sig_o")
            for c in range(NCHUNK):
                nc.gpsimd.indirect_dma_start(
                    out=sig_o[NORPH * c:NORPH * (c + 1), :], out_offset=None,
                    in_=table_d[:, :],
                    in_offset=bass.IndirectOffsetOnAxis(ap=idxo[:, 0:1], axis=0),
                    element_offset=c * W,
                )
            fe_o = opool.tile([NORPH_T, PS], BF16, tag="fe_o")
            nc.sync.dma_start(out=fe_o[:, :], in_=feato_d[:, :])

            # ---- per-stream indices ----
            idx_tiles = []
            for j in range(4):
                nc.vector.tensor_scalar(
                    out=ge[:, :], in0=thr[:, :], scalar1=x4[:, j:j + 1],
                    scalar2=None, op0=mybir.AluOpType.is_le)
                nc.vector.tensor_reduce(out=idxf[:, :], in_=ge[:, :],
                                        axis=mybir.AxisListType.X,
                                        op=mybir.AluOpType.add)
                it = cpool.tile([128, 1], I32, tag=f"idx{j}")
                nc.vector.tensor_copy(out=it[:, :], in_=idxf[:, :])
                idx_tiles.append(it)

            # ---- orphan bind / U / Q, once for all chunks ----
            nc.vector.tensor_tensor(out=sig_o[:, :], in0=sig_o[:, :],
                                    in1=fe_o[:, :], op=mybir.AluOpType.mult)
            u_src = opool.tile([NORPH_T, WU], BF16, tag="u_src")
            nc.scalar.dma_start(out=u_src[0:NORPH_T - 1, :],
                                in_=sig_o[1:NORPH_T, 1:1 + WU])
            u_o = opool.tile([NORPH_T, WU], BF16, tag="u_o")
            nc.vector.tensor_tensor(out=u_o[0:NORPH_T - 1, :],
                                    in0=sig_o[0:NORPH_T - 1, 0:WU],
                                    in1=u_src[0:NORPH_T - 1, :],
                                    op=mybir.AluOpType.mult)
            u1_src = opool.tile([NORPH_T, W], BF16, tag="u1_src")
            nc.scalar.dma_start(out=u1_src[0:NORPH_T - 3, :],
                                in_=u_o[2:NORPH_T - 1, 2:2 + W])
            q_o = opool.tile([NORPH_T, W], BF16, tag="q_o")
            for k in range(3):  # rows 27..29 stay unwritten; zero for E-matmul
                nc.sync.dma_start(out=q_o[NORPH_T - 3 + k:NORPH_T - 2 + k, :],
                                  in_=zrow_d[0:1, 0:W])
            nc.vector.tensor_tensor(out=q_o[0:NORPH_T - 3, :],
                                    in0=u_o[0:NORPH_T - 3, 0:W],
                                    in1=u1_src[0:NORPH_T - 3, :],
                                    op=mybir.AluOpType.mult)

            done_slices = 0
            for c in range(NCHUNK):
                c0 = c * W

                # ---------- packed loads ----------
                sig4 = lpool.tile([128, 4 * PS], BF16, tag="sig4")
                for j in range(4):
                    nc.gpsimd.indirect_dma_start(
                        out=sig4[:, j * PS:(j + 1) * PS], out_offset=None,
                        in_=table_d[:, :],
                        in_offset=bass.IndirectOffsetOnAxis(
                            ap=idx_tiles[j][:, 0:1], axis=0),
                        element_offset=c0,
                    )
                fe4 = lpool.tile([128, 4 * PS], BF16, tag="fe4")
                nc.sync.dma_start(out=fe4[:, :], in_=feat4_d[c, :, :])

                # ---------- packed S (in place) ----------
                sig4_r = sig4[:, :].rearrange("p (s w) -> p s w", s=4)
                nc.vector.tensor_tensor(out=sig4[:, :], in0=sig4[:, :],
                                        in1=fe4[:, :], op=mybir.AluOpType.mult)

                # A'[p] = S_0[p+1, 1:2003]  (split SBUF->SBUF DMA)
                a1 = a1_raw[c % 2]
                qs = [nc.sync, nc.scalar]
                for k in range(8):
                    n = 16 if k < 7 else 15
                    qs[k % 2].dma_start(
                        out=a1[16 * k:16 * k + n, :],
                        in_=sig4[16 * k + 1:16 * k + 1 + n, 1:1 + WU])

                # ---------- packed U (strided ops) ----------
                u4 = wpool.tile([128, 4 * UP], BF16, tag="u4")
                u4_r = u4[:, :].rearrange("p (s w) -> p s w", s=4)
                nc.vector.tensor_tensor(
                    out=u4_r[:, 0:3, 0:WU],
                    in0=sig4_r[:, 0:3, 0:WU],
                    in1=sig4_r[:, 1:4, 1:1 + WU],
                    op=mybir.AluOpType.mult)
                nc.vector.tensor_tensor(
                    out=u4[:, 3 * UP:3 * UP + WU],
                    in0=sig4[:, 3 * PS:3 * PS + WU],
                    in1=a1[:, :],
                    op=mybir.AluOpType.mult)

                # ---------- V = U_{0,1}[p+1, +2] via shift matmul ----------
                v4 = vpool.tile([128, 2 * W], BF16, tag="v4")
                for s in range(2):
                    for a0, a1s in SEGS:
                        vp = pv.tile([128, 512], F32, tag="vp")
                        nc.tensor.matmul(
                            out=vp[:, 0:a1s - a0],
                            lhsT=sh1[:, :],
                            rhs=u4[:, s * UP + 2 + a0:s * UP + 2 + a1s],
                            start=True, stop=True)
                        nc.scalar.copy(out=v4[:, s * W + a0:s * W + a1s],
                                       in_=vp[:, 0:a1s - a0])

                # ---------- packed Q (in place over u4) ----------
                nc.vector.tensor_tensor(
                    out=u4_r[:, 0:2, 0:W],
                    in0=u4_r[:, 0:2, 0:W],
                    in1=u4_r[:, 2:4, 2:2 + W],
                    op=mybir.AluOpType.mult)
                v4_r = v4[:, :].rearrange("p (s w) -> p s w", s=2)
                nc.vector.tensor_tensor(
                    out=u4_r[:, 2:4, 0:W],
                    in0=u4_r[:, 2:4, 0:W],
                    in1=v4_r[:, :, :],
                    op=mybir.AluOpType.mult)

                # ---------- bundle reduce (packed + orphan) ----------
                # Q2-dependent matmuls last so only they trail the Q2 op
                accp = pacc.tile([1, W], F32, tag="acc")
                for a0, a1s in SEGS:
                    for j in range(2):
                        nc.tensor.matmul(out=accp[0:1, a0:a1s],
                                         lhsT=onr[:, 0:1],
                                         rhs=u4[:, j * UP + a0:j * UP + a1s],
                                         start=(j == 0), stop=False)
                    nc.tensor.matmul(out=accp[0:1, a0:a1s],
                                     lhsT=eorph[:, c:c + 1],
                                     rhs=q_o[:, a0:a1s],
                                     start=False, stop=False)
                    for j in range(2, 4):
                        nc.tensor.matmul(out=accp[0:1, a0:a1s],
                                         lhsT=onr[:, 0:1],
                                         rhs=u4[:, j * UP + a0:j * UP + a1s],
                                         start=False, stop=(j == 3))
                stg = fpool.tile([1, W], F32, tag="stg")
                nc.scalar.copy(out=stg[:, :], in_=accp[0:1, :])
                nc.sync.dma_start(out=cc_in[0:1, c0:c0 + W], in_=stg[:, :])

            # All AllReduces after the loop: each trigger still fires as
            # soon as its own slice's inputs land, but a waiting trigger
            # can no longer block gather dispatches on the gpsimd queue.
            for s0a, s1a, _ in AR_SLICES:
                nc.gpsimd.collective_compute(
                    "AllReduce", mybir.AluOpType.add,
                    ins=[cc_in[0:1, s0a:s1a]], outs=[cc_out[0:1, s0a:s1a]],
                    replica_groups=[list(range(NCORE))],
                )

            # ---- sign + roll-by-3 output per slice, all at the end ----
            for s0, s1, _ in AR_SLICES:
                npr = (s1 - s0) // 80
                r = fpool.tile([75, 80], F32, tag="fin")
                nc.sync.dma_start(
                    out=r[0:npr, :],
                    in_=cc_out[0:1, s0:s1].rearrange("o (p w) -> (o p) w",
                                                     p=npr))
                t1 = fpool.tile([75, 80], F32, tag="fin2")
                nc.vector.tensor_scalar(
                    out=t1[0:npr, :], in0=r[0:npr, :], scalar1=0.0,
                    scalar2=2.0, op0=mybir.AluOpType.is_gt,
                    op1=mybir.AluOpType.mult)
                sg = fpool.tile([75, 80], F32, tag="fin3")
                nc.vector.tensor_scalar(
                    out=sg[0:npr, :], in0=t1[0:npr, :], scalar1=-1.0,
                    scalar2=None, op0=mybir.AluOpType.add)
                if s1 < D:
                    nc.sync.dma_start(out=out_d[0:1, s0 + ROLL:s1 + ROLL],
                                      in_=sg[0:npr, :])
                else:
                    nfull = (s1 - s0 - ROLL) // 80       # 24 full rows
                    rem = (s1 - s0 - ROLL) - nfull * 80  # 77
                    nc.sync.dma_start(
                        out=out_d[0:1, s0 + ROLL:s0 + ROLL + nfull * 80],
                        in_=sg[0:nfull, :])
                    nc.sync.dma_start(
                        out=out_d[0:1, s0 + ROLL + nfull * 80:D],
                        in_=sg[nfull:nfull + 1, 0:rem])
                    nc.sync.dma_start(
                        out=out_d[0:1, 0:ROLL],
                        in_=sg[nfull:nfull + 1, rem:80])

    nc.compile()
    return nc


TRACE = False
LAST_RESULT = None


def _pad_rows(fw, base, n):
    """rows [base, base+n) of fw, zero-padded past NFEAT, with DPAD wrap."""
    out = np.zeros((n, DPAD), dtype=_BF)
    nreal = max(0, min(n, NFEAT - base))
    if nreal > 0:
        fb = fw[base:base + nreal].astype(_BF)
        out[:nreal, :D] = fb
        out[:nreal, D:] = fb[:, :DPAD - D]
    return out


def _make_in_maps(xf, sw, fw, consts):
    table = np.empty((NUM_LEVELS, DPAD), dtype=_BF)
    table[:, :D] = sw.astype(_BF)
    table[:, D:] = table[:, :DPAD - D]

    in_maps = []
    for m in range(NCORE):
        base = PER_CORE * m

        # packed feat: feat4[c, p, j*PS + e] = fw_pad[base + 4p + j, c*W + e]
        fp = _pad_rows(fw, base, PER_CORE)              # (512, DPAD)
        fp4 = np.zeros((NCHUNK, 128, 4 * PS), dtype=_BF)
        for c in range(NCHUNK):
            sl = fp[:, c * W:c * W + PS]                # (512, PS)
            fp4[c] = sl.reshape(128, 4 * PS)

        # orphan feat rows base+509 .. base+514, chunk-major (6c+r)
        fo = _pad_rows(fw, base + PER_CORE - 3, NORPH)  # (6, DPAD)
        fo5 = np.zeros((NORPH_T, PS), dtype=_BF)
        for c in range(NCHUNK):
            fo5[NORPH * c:NORPH * (c + 1)] = fo[:, c * W:c * W + PS]

        xr = np.full(PER_CORE + NORPH, xf[-1], dtype=np.float32)
        nreal = min(PER_CORE + 3, NFEAT - base)
        xr[:nreal] = xf[base:base + nreal]
        x4 = xr[:PER_CORE].reshape(128, 4).copy()
        xo = xr[PER_CORE - 3:PER_CORE - 3 + NORPH].reshape(NORPH, 1).copy()

        in_maps.append({
            "x4": x4,
            "x_orph": xo,
            "thr": consts["thr"],
            "table": table,
            "feat4": fp4,
            "feat_orph": fo5,
            "sh1": consts["sh1"],
            "eorph": consts["eorph"],
            "ones_red": consts["ones_red"],
            "zrow": consts["zrow"],
        })
    return in_maps


def kernel(x, signals_weight, feat_weight):
    global LAST_RESULT
    consts = _host_constants()

    if "nc" not in _CACHE:
        _CACHE["nc"] = _build_program()
    nc = _CACHE["nc"]

    xf = np.asarray(x, dtype=np.float32).reshape(-1)
    sw = np.asarray(signals_weight, dtype=np.float32)
    fw = np.asarray(feat_weight, dtype=np.float32)
    in_maps = _make_in_maps(xf, sw, fw, consts)

    res = run_bass_kernel_spmd(nc, in_maps, list(range(NCORE)), trace=TRACE)
    LAST_RESULT = res
    return np.asarray(res.results[0]["out"], dtype=np.float32)


# revision 55
# speedup vs baseline: 1.1701x; 1.1701x over previous
"""Trainium2 Bass kernel for nn_FChCombxValEncoder (HDC n-gram encoder).

Computation: idx = quantize(x) -> signal = signals_weight[idx] -> bind with
feat_weight -> 4-gram product with per-step D-rolls -> bundle sum -> sign.

Distribution: feature axis (4096) sharded across 8 cores (512 n-gram starts
each); per-core bundle partials are AllReduced in 2 D-slices emitted after
the chunk loop: each trigger fires as soon as its own slice's inputs land,
pipelining the first collective behind the last chunks' compute (a single
end-of-kernel AllReduce costs ~47us exposed; this costs ~25us).

Layout: each partition p holds FOUR consecutive rows t = 4p+j as four
streams side by side in the free dim (stream pitch 2005).  Per 2000-wide
D-chunk:
    S_j = sig_j . feat_j          one strided in-place bind op (bf16 2x)
    U_j = S_j . S_{j+1}(+1)       j=0..2 one strided op; j=3 uses A1 =
                                  S_0 of partition p+1 via split SBUF DMA
    Q_j = U_j . U_{j+2}(+2)       two strided in-place ops; j=2,3 use
                                  V = U_{0,1}[p+1] via shift-matmul on PE
                                  (zero col 127 => exact for +/-1)
    R   = ones^T @ Q              PSUM accumulate, 512-col segs
The 3 grams per core needing rows past the 512-row pack ("orphans") are
computed ONCE for all 5 chunks on 30 partitions (6 rows x 5 chunks); the
per-chunk selection happens in the reduce matmul via a 30x5 0/1 matrix E
(garbage rows weighted 0; unwritten rows zeroed once).

Sign+output for each slice is emitted only after the chunk loop so its
AllReduce-wait never blocks the Vector queue ahead of chunk work; the
AllReduce triggers are all emitted after the loop too, because a trigger
waiting anywhere inside the loop stalls the in-order GpSimd queue ahead
of gather dispatches (measured 20-40us of lost overlap).

All values are +/-1 so bf16 is exact; bundle partials are integers < 2^24
so fp32 PSUM/AllReduce is exact; the output sign never sees zero (4093 odd
terms).  Index quantization is bit-exact via a host fp32 threshold table:
idx = #{k : x >= b_k}.
"""
import sys

sys.path.insert(0, "/opt/trn_rl_repo")

import numpy as np
import ml_dtypes

import concourse.bass as bass
import concourse.bacc as bacc
import concourse.tile as tile
import concourse.mybir as mybir
from concourse.bass_utils import run_bass_kernel_spmd

# ---- problem constants ----
MAX_VAL = 52000.0
MIN_VAL = -53000.0
NUM_LEVELS = 1000
NGRAM = 4
D = 10000
NFEAT = 4096
NCORE = 8

PER_CORE = 512
ROLL = NGRAM - 1

W = 2000                      # D-chunk width
NCHUNK = D // W               # 5
PS = 2005                     # packed stream pitch
WB = 2003                     # S cols needed per chunk (U reach)
UP = 2004                     # U stream pitch (even), data width 2002
WU = 2002                     # U stream data width
DPAD = D + 5                  # table/orphan row width 10005
NORPH = 6                     # orphan rows per core (t = 509..514)
NORPH_T = NCHUNK * NORPH      # 30: orphan rows unrolled over chunks
SEGS = [(0, 512), (512, 1024), (1024, 1536), (1536, 2000)]
# AllReduce D-slices: (start, end, last chunk needed)
AR_SLICES = [(0, 6000, 2), (6000, 10000, 4)]

F32 = mybir.dt.float32
BF16 = mybir.dt.bfloat16
I32 = mybir.dt.int32
_BF = ml_dtypes.bfloat16


# ---------------------------------------------------------------- host prep
def _f2o(u):
    b = u.view(np.uint32).astype(np.int64)
    return np.where(b < 0x80000000, b + 0x80000000, 0xFFFFFFFF - b)


def _o2f(o):
    b = np.where(o >= 0x80000000, o - 0x80000000, 0xFFFFFFFF - o).astype(np.uint64)
    return b.astype(np.uint32).view(np.float32)


def _g(v):
    v = v.astype(np.float32)
    t = (v - np.float32(MIN_VAL)).astype(np.float32)
    t = (t / np.float32(MAX_VAL - MIN_VAL)).astype(np.float32)
    t = (t * np.float32(NUM_LEVELS - 1)).astype(np.float32)
    return np.clip(np.round(t), 0.0, float(NUM_LEVELS - 1))


def _thresholds():
    ks = np.arange(1, NUM_LEVELS, dtype=np.float32)
    lo = _f2o(np.full(ks.shape, np.float32(MIN_VAL) - np.float32(2.0)))
    hi = _f2o(np.full(ks.shape, np.float32(MAX_VAL) + np.float32(2.0)))
    for _ in range(64):
        mid = (lo + hi) // 2
        ge = _g(_o2f(mid)) >= ks
        hi = np.where(ge, mid, hi)
        lo = np.where(ge, lo, mid)
        if np.all(hi - lo <= 1):
            break
    return _o2f(hi)


_CACHE = {}


def _host_constants():
    if "thr" not in _CACHE:
        _CACHE["thr"] = np.tile(_thresholds()[None, :], (128, 1)).astype(np.float32)
        sh1 = np.zeros((128, 128), dtype=_BF)
        for i in range(127):
            sh1[i + 1, i] = 1.0          # V[m] = U[m+1]; col 127 zero
        _CACHE["sh1"] = sh1
        _CACHE["ones_red"] = np.ones((128, 1), dtype=_BF)
        _CACHE["zrow"] = np.zeros((1, PS), dtype=_BF)
        eo = np.zeros((NORPH_T, NCHUNK), dtype=_BF)
        for c in range(NCHUNK):
            eo[NORPH * c:NORPH * c + 3, c] = 1.0
        _CACHE["eorph"] = eo
    return _CACHE


# ---------------------------------------------------------------- program
def _build_program():
    nc = bacc.Bacc("TRN2", target_bir_lowering=False, debug=False,
                   num_devices=NCORE)

    x4_d = nc.dram_tensor("x4", (128, 4), F32, kind="ExternalInput")
    xo_d = nc.dram_tensor("x_orph", (NORPH, 1), F32, kind="ExternalInput")
    thr_d = nc.dram_tensor("thr", (128, NUM_LEVELS - 1), F32, kind="ExternalInput")
    table_d = nc.dram_tensor("table", (NUM_LEVELS, DPAD), BF16, kind="ExternalInput")
    feat4_d = nc.dram_tensor("feat4", (NCHUNK, 128, 4 * PS), BF16, kind="ExternalInput")
    feato_d = nc.dram_tensor("feat_orph", (NORPH_T, PS), BF16, kind="ExternalInput")
    sh1_d = nc.dram_tensor("sh1", (128, 128), BF16, kind="ExternalInput")
    eorph_d = nc.dram_tensor("eorph", (NORPH_T, NCHUNK), BF16, kind="ExternalInput")
    onr_d = nc.dram_tensor("ones_red", (128, 1), BF16, kind="ExternalInput")
    zrow_d = nc.dram_tensor("zrow", (1, PS), BF16, kind="ExternalInput")
    out_d = nc.dram_tensor("out", (1, D), F32, kind="ExternalOutput")

    cc_in = nc.dram_tensor("cc_in", (1, D), F32)
    cc_out = nc.dram_tensor("cc_out", (1, D), F32, addr_space="Shared")

    NTH = NUM_LEVELS - 1

    # raw double-buffered A' tensors; row 127 kept zero
    a1_raw = [nc.alloc_sbuf_tensor(f"a1f{i}", [128, WU], BF16).ap()
              for i in range(2)]

    with tile.TileContext(nc) as tc:
        with tc.tile_pool(name="const", bufs=1) as cpool, \
             tc.tile_pool(name="scr", bufs=1) as spool_s, \
             tc.tile_pool(name="loads", bufs=2) as lpool, \
             tc.tile_pool(name="work", bufs=2) as wpool, \
             tc.tile_pool(name="orph", bufs=1) as opool, \
             tc.tile_pool(name="vone", bufs=1) as vpool, \
             tc.tile_pool(name="fin", bufs=1) as fpool, \
             tc.tile_pool(name="pv", bufs=4, space="PSUM") as pv, \
             tc.tile_pool(name="pacc", bufs=1, space="PSUM") as pacc:

            # ---- const loads; thr split across two queues ----
            thr = cpool.tile([128, NTH], F32)
            for k in range(2):
                [nc.sync, nc.scalar][k].dma_start(
                    out=thr[64 * k:64 * (k + 1), :],
                    in_=thr_d[64 * k:64 * (k + 1), :])
            sh1 = cpool.tile([128, 128], BF16)
            nc.sync.dma_start(out=sh1[:, :], in_=sh1_d[:, :])
            onr = cpool.tile([128, 1], BF16)
            nc.sync.dma_start(out=onr[:, :], in_=onr_d[:, :])
            eorph = cpool.tile([NORPH_T, NCHUNK], BF16)
            nc.sync.dma_start(out=eorph[:, :], in_=eorph_d[:, :])
            x4 = cpool.tile([128, 4], F32)
            nc.sync.dma_start(out=x4[:, :], in_=x4_d[:, :])
            xo = cpool.tile([NORPH, 1], F32)
            nc.scalar.dma_start(out=xo[:, :], in_=xo_d[:, :])

            for a1 in a1_raw:
                nc.sync.dma_start(out=a1[127:128, 0:WU], in_=zrow_d[0:1, 0:WU])

            # ---- orphan indices first so orphan gathers dispatch early ----
            ge = spool_s.tile([128, NTH], BF16, tag="ge")
            idxf = spool_s.tile([128, 1], F32, tag="idxf")
            nc.vector.tensor_scalar(
                out=ge[0:NORPH, :], in0=thr[0:NORPH, :], scalar1=xo[:, 0:1],
                scalar2=None, op0=mybir.AluOpType.is_le)
            nc.vector.tensor_reduce(out=idxf[0:NORPH, :], in_=ge[0:NORPH, :],
                                    axis=mybir.AxisListType.X,
                                    op=mybir.AluOpType.add)
            idxo = cpool.tile([NORPH, 1], I32, tag="idxo")
            nc.vector.tensor_copy(out=idxo[:, :], in_=idxf[0:NORPH, :])

            # ---- orphan gathers (6 rows x 5 chunks, chunk-major) ----
            sig_o = opool.tile([NORPH_T, PS], BF16, tag="sig_o")
            for c in range(NCHUNK):
                nc.gpsimd.indirect_dma_start(
                    out=sig_o[NORPH * c:NORPH * (c + 1), :], out_offset=None,
                    in_=table_d[:, :],
                    in_offset=bass.IndirectOffsetOnAxis(ap=idxo[:, 0:1], axis=0),
                    element_offset=c * W,
                )
            fe_o = opool.tile([NORPH_T, PS], BF16, tag="fe_o")
            nc.sync.dma_start(out=fe_o[:, :], in_=feato_d[:, :])

            # ---- per-stream indices ----
            idx_tiles = []
            for j in range(4):
                nc.vector.tensor_scalar(
                    out=ge[:, :], in0=thr[:, :], scalar1=x4[:, j:j + 1],
                    scalar2=None, op0=mybir.AluOpType.is_le)
                nc.vector.tensor_reduce(out=idxf[:, :], in_=ge[:, :],
                                        axis=mybir.AxisListType.X,
                                        op=mybir.AluOpType.add)
                it = cpool.tile([128, 1], I32, tag=f"idx{j}")
                nc.vector.tensor_copy(out=it[:, :], in_=idxf[:, :])
                idx_tiles.append(it)

            # ---- orphan bind / U / Q, once for all chunks ----
            nc.vector.tensor_tensor(out=sig_o[:, :], in0=sig_o[:, :],
                                    in1=fe_o[:, :], op=mybir.AluOpType.mult)
            u_src = opool.tile([NORPH_T, WU], BF16, tag="u_src")
            nc.scalar.dma_start(out=u_src[0:NORPH_T - 1, :],
                                in_=sig_o[1:NORPH_T, 1:1 + WU])
            u_o = opool.tile([NORPH_T, WU], BF16, tag="u_o")
            nc.vector.tensor_tensor(out=u_o[0:NORPH_T - 1, :],
                                    in0=sig_o[0:NORPH_T - 1, 0:WU],
                                    in1=u_src[0:NORPH_T - 1, :],
                                    op=mybir.AluOpType.mult)
            u1_src = opool.tile([NORPH_T, W], BF16, tag="u1_src")
            nc.scalar.dma_start(out=u1_src[0:NORPH_T - 3, :],
                                in_=u_o[2:NORPH_T - 1, 2:2 + W])
            q_o = opool.tile([NORPH_T, W], BF16, tag="q_o")
            for k in range(3):  # rows 27..29 stay unwritten; zero for E-matmul
                nc.sync.dma_start(out=q_o[NORPH_T - 3 + k:NORPH_T - 2 + k, :],
                                  in_=zrow_d[0:1, 0:W])
            nc.vector.tensor_tensor(out=q_o[0:NORPH_T - 3, :],
                                    in0=u_o[0:NORPH_T - 3, 0:W],
                                    in1=u1_src[0:NORPH_T - 3, :],
                                    op=mybir.AluOpType.mult)

            done_slices = 0
            for c in range(NCHUNK):
                c0 = c * W

                # ---------- packed loads ----------
                sig4 = lpool.tile([128, 4 * PS], BF16, tag="sig4")
                for j in range(4):
                    nc.gpsimd.indirect_dma_start(
                        out=sig4[:, j * PS:(j + 1) * PS], out_offset=None,
                        in_=table_d[:, :],
                        in_offset=bass.IndirectOffsetOnAxis(
                            ap=idx_tiles[j][:, 0:1], axis=0),
                        element_offset=c0,
                    )
                fe4 = lpool.tile([128, 4 * PS], BF16, tag="fe4")
                nc.sync.dma_start(out=fe4[:, :], in_=feat4_d[c, :, :])

                # ---------- packed S (in place) ----------
                sig4_r = sig4[:, :].rearrange("p (s w) -> p s w", s=4)
                nc.vector.tensor_tensor(out=sig4[:, :], in0=sig4[:, :],
                                        in1=fe4[:, :], op=mybir.AluOpType.mult)

                # A'[p] = S_0[p+1, 1:2003]  (split SBUF->SBUF DMA)
                a1 = a1_raw[c % 2]
                qs = [nc.sync, nc.scalar, nc.gpsimd]
                for k in range(8):
                    n = 16 if k < 7 else 15
                    qs[k % 3].dma_start(
                        out=a1[16 * k:16 * k + n, :],
                        in_=sig4[16 * k + 1:16 * k + 1 + n, 1:1 + WU])

                # ---------- packed U (strided ops) ----------
                u4 = wpool.tile([128, 4 * UP], BF16, tag="u4")
                u4_r = u4[:, :].rearrange("p (s w) -> p s w", s=4)
                nc.vector.tensor_tensor(
                    out=u4_r[:, 0:3, 0:WU],
                    in0=sig4_r[:, 0:3, 0:WU],
                    in1=sig4_r[:, 1:4, 1:1 + WU],
                    op=mybir.AluOpType.mult)
                nc.vector.tensor_tensor(
                    out=u4[:, 3 * UP:3 * UP + WU],
                    in0=sig4[:, 3 * PS:3 * PS + WU],
                    in1=a1[:, :],
                    op=mybir.AluOpType.mult)

                # ---------- V = U_{0,1}[p+1, +2] via shift matmul ----------
                v4 = vpool.tile([128, 2 * W], BF16, tag="v4")
                for s in range(2):
                    for a0, a1s in SEGS:
                        vp = pv.tile([128, 512], F32, tag="vp")
                        nc.tensor.matmul(
                            out=vp[:, 0:a1s - a0],
                            lhsT=sh1[:, :],
                            rhs=u4[:, s * UP + 2 + a0:s * UP + 2 + a1s],
                            start=True, stop=True)
                        nc.scalar.copy(out=v4[:, s * W + a0:s * W + a1s],
                                       in_=vp[:, 0:a1s - a0])

                # ---------- packed Q (in place over u4) ----------
                nc.vector.tensor_tensor(
                    out=u4_r[:, 0:2, 0:W],
                    in0=u4_r[:, 0:2, 0:W],
                    in1=u4_r[:, 2:4, 2:2 + W],
                    op=mybir.AluOpType.mult)
                v4_r = v4[:, :].rearrange("p (s w) -> p s w", s=2)
                nc.vector.tensor_tensor(
                    out=u4_r[:, 2:4, 0:W],
                    in0=u4_r[:, 2:4, 0:W],
                    in1=v4_r[:, :, :],
                    op=mybir.AluOpType.mult)

                # ---------- bundle reduce (packed + orphan) ----------
                # Q2-dependent matmuls last so only they trail the Q2 op
                accp = pacc.tile([1, W], F32, tag="acc")
                for a0, a1s in SEGS:
                    for j in range(2):
                        nc.tensor.matmul(out=accp[0:1, a0:a1s],
                                         lhsT=onr[:, 0:1],
                                         rhs=u4[:, j * UP + a0:j * UP + a1s],
                                         start=(j == 0), stop=False)
                    nc.tensor.matmul(out=accp[0:1, a0:a1s],
                                     lhsT=eorph[:, c:c + 1],
                                     rhs=q_o[:, a0:a1s],
                                     start=False, stop=False)
                    for j in range(2, 4):
                        nc.tensor.matmul(out=accp[0:1, a0:a1s],
                                         lhsT=onr[:, 0:1],
                                         rhs=u4[:, j * UP + a0:j * UP + a1s],
                                         start=False, stop=(j == 3))
                stg = fpool.tile([1, W], F32, tag="stg")
                nc.scalar.copy(out=stg[:, :], in_=accp[0:1, :])
                nc.sync.dma_start(out=cc_in[0:1, c0:c0 + W], in_=stg[:, :])

            # All AllReduces after the loop: each trigger still fires as
            # soon as its own slice's inputs land, but a waiting trigger
            # can no longer block gather dispatches on the gpsimd queue.
            for s0a, s1a, _ in AR_SLICES:
                nc.gpsimd.collective_compute(
                    "AllReduce", mybir.AluOpType.add,
                    ins=[cc_in[0:1, s0a:s1a]], outs=[cc_out[0:1, s0a:s1a]],
                    replica_groups=[list(range(NCORE))],
                )

            # ---- sign + roll-by-3 output per slice, all at the end ----
            for s0, s1, _ in AR_SLICES:
                npr = (s1 - s0) // 80
                r = fpool.tile([75, 80], F32, tag="fin")
                nc.sync.dma_start(
                    out=r[0:npr, :],
                    in_=cc_out[0:1, s0:s1].rearrange("o (p w) -> (o p) w",
                                                     p=npr))
                t1 = fpool.tile([75, 80], F32, tag="fin2")
                nc.vector.tensor_scalar(
                    out=t1[0:npr, :], in0=r[0:npr, :], scalar1=0.0,
                    scalar2=2.0, op0=mybir.AluOpType.is_gt,
                    op1=mybir.AluOpType.mult)
                sg = fpool.tile([75, 80], F32, tag="fin3")
                nc.vector.tensor_scalar(
                    out=sg[0:npr, :], in0=t1[0:npr, :], scalar1=-1.0,
                    scalar2=None, op0=mybir.AluOpType.add)
                if s1 < D:
                    nc.sync.dma_start(out=out_d[0:1, s0 + ROLL:s1 + ROLL],
                                      in_=sg[0:npr, :])
                else:
                    nfull = (s1 - s0 - ROLL) // 80       # 24 full rows
                    rem = (s1 - s0 - ROLL) - nfull * 80  # 77
                    nc.sync.dma_start(
                        out=out_d[0:1, s0 + ROLL:s0 + ROLL + nfull * 80],
                        in_=sg[0:nfull, :])
                    nc.sync.dma_start(
                        out=out_d[0:1, s0 + ROLL + nfull * 80:D],
                        in_=sg[nfull:nfull + 1, 0:rem])
                    nc.sync.dma_start(
                        out=out_d[0:1, 0:ROLL],
                        in_=sg[nfull:nfull + 1, rem:80])

    nc.compile()
    return nc


TRACE = False
LAST_RESULT = None


def _pad_rows(fw, base, n):
    """rows [base, base+n) of fw, zero-padded past NFEAT, with DPAD wrap."""
    out = np.zeros((n, DPAD), dtype=_BF)
    nreal = max(0, min(n, NFEAT - base))
    if nreal > 0:
        fb = fw[base:base + nreal].astype(_BF)
        out[:nreal, :D] = fb
        out[:nreal, D:] = fb[:, :DPAD - D]
    return out


def _make_in_maps(xf, sw, fw, consts):
    table = np.empty((NUM_LEVELS, DPAD), dtype=_BF)
    table[:, :D] = sw.astype(_BF)
    table[:, D:] = table[:, :DPAD - D]

    in_maps = []
    for m in range(NCORE):
        base = PER_CORE * m

        # packed feat: feat4[c, p, j*PS + e] = fw_pad[base + 4p + j, c*W + e]
        fp = _pad_rows(fw, base, PER_CORE)              # (512, DPAD)
        fp4 = np.zeros((NCHUNK, 128, 4 * PS), dtype=_BF)
        for c in range(NCHUNK):
            sl = fp[:, c * W:c * W + PS]                # (512, PS)
            fp4[c] = sl.reshape(128, 4 * PS)

        # orphan feat rows base+509 .. base+514, chunk-major (6c+r)
        fo = _pad_rows(fw, base + PER_CORE - 3, NORPH)  # (6, DPAD)
        fo5 = np.zeros((NORPH_T, PS), dtype=_BF)
        for c in range(NCHUNK):
            fo5[NORPH * c:NORPH * (c + 1)] = fo[:, c * W:c * W + PS]

        xr = np.full(PER_CORE + NORPH, xf[-1], dtype=np.float32)
        nreal = min(PER_CORE + 3, NFEAT - base)
        xr[:nreal] = xf[base:base + nreal]
        x4 = xr[:PER_CORE].reshape(128, 4).copy()
        xo = xr[PER_CORE - 3:PER_CORE - 3 + NORPH].reshape(NORPH, 1).copy()

        in_maps.append({
            "x4": x4,
            "x_orph": xo,
            "thr": consts["thr"],
            "table": table,
            "feat4": fp4,
            "feat_orph": fo5,
            "sh1": consts["sh1"],
            "eorph": consts["eorph"],
            "ones_red": consts["ones_red"],
            "zrow": consts["zrow"],
        })
    return in_maps


def kernel(x, signals_weight, feat_weight):
    global LAST_RESULT
    consts = _host_constants()

    if "nc" not in _CACHE:
        _CACHE["nc"] = _build_program()
    nc = _CACHE["nc"]

    xf = np.asarray(x, dtype=np.float32).reshape(-1)
    sw = np.asarray(signals_weight, dtype=np.float32)
    fw = np.asarray(feat_weight, dtype=np.float32)
    in_maps = _make_in_maps(xf, sw, fw, consts)

    res = run_bass_kernel_spmd(nc, in_maps, list(range(NCORE)), trace=TRACE)
    LAST_RESULT = res
    return np.asarray(res.results[0]["out"], dtype=np.float32)
